# revision 13
# baseline (speedup 1.0000x reference)
"""Trainium2 Bass kernel for nn_CombineModel_wo_net (histogram_binning).

Full inputs in, full output out. Internally: data-parallel across 8
NeuronCores, 2 images per core. Each core streams its 2x3x544x960 fp32
slice from HBM and reduces it to per-partition partials:
  - sum of s = c0+c1+c2 per pixel  (for avg brightness)
  - count(s >= 2.25)               (bright pixels, g >= 0.75)
  - count(s >= 0.75)               (complement of dark: g >= 0.25)
The tiny [5,16] epilogue (dynamic-range ratio, gap select, exposure
where-chains) is replicated exactly in float32 numpy on the host from
the gathered partials.

Threshold equivalence note: comparing s = c0+c1+c2 against 3*T is exact
w.r.t. the reference's g = mean(c) >= T because fp32 spacing at s~3T is
wider than the rounding interval of s/3 (or s*(1/3)) around T for
T in {0.25, 0.75}; no representable s straddles the thresholds.
"""

import sys

for _p in ("/opt/trn_rl_repo",):
    if _p not in sys.path:
        sys.path.insert(0, _p)

from contextlib import ExitStack

import numpy as np

import concourse.bass as bass
import concourse.bacc as bacc
import concourse.mybir as mybir
import concourse.tile as tile
from concourse.bass_utils import run_bass_kernel_spmd

# Problem geometry (hardcoded per contract).
B, C, H, W = 16, 3, 544, 960
N_CORES = 8
IMGS_PER_CORE = B // N_CORES          # 2
PLANE = H * W                          # 522240 = 128 * 4080
P = 128
COLS = PLANE // P                      # 4080
CHUNK = 1360                           # plane split into 3 chunks for DMA/compute overlap
NCHUNK = COLS // CHUNK                 # 3
NQ = 3                                 # sum_s, cnt_ge_2.25, cnt_ge_0.75
NACC = IMGS_PER_CORE * NCHUNK * NQ     # 18 accumulator columns

F32 = mybir.dt.float32

# Module-level knobs (test.py pokes these; grading path uses defaults).
TRACE = False
LAST_RESULT = None  # BassKernelResults of most recent run (for profiling)

_compiled_nc = None


def _build_bass(reps=1, body_copies=1, chunk=CHUNK, in_bufs=7, tmp_bufs=5,
                bits_bufs=5, emit_counts=True, dma_alt=False, fused_dma=False,
                dma_accum=False):
    """Emit the per-core Tile program (same SPMD program on all 8 cores).

    reps > 1 wraps the workload in a hardware For_i loop so one NEFF
    execution runs it `reps * body_copies` times; the bench harness uses
    marginal time per iteration as the HW exec time. The grading path
    uses reps=1, body_copies=1 (no loop).
    """
    nchunk = COLS // chunk
    nacc = IMGS_PER_CORE * nchunk * NQ
    nc = bacc.Bacc(
        "TRN2", target_bir_lowering=False, debug=False, num_devices=N_CORES
    )
    img = nc.dram_tensor(
        "img", [IMGS_PER_CORE, C, P, COLS], F32, kind="ExternalInput"
    ).ap()
    acc_out = nc.dram_tensor("acc", [P, nacc], F32, kind="ExternalOutput").ap()

    add = mybir.AluOpType.add
    is_ge = mybir.AluOpType.is_ge

    with ExitStack() as ctx:
        tc = ctx.enter_context(tile.TileContext(nc))
        pool_in = ctx.enter_context(tc.tile_pool(name="inp", bufs=in_bufs))
        pool_tmp = ctx.enter_context(tc.tile_pool(name="tmp", bufs=tmp_bufs))
        pool_bits = ctx.enter_context(
            tc.tile_pool(name="bitsp", bufs=bits_bufs or tmp_bufs)
        )
        pool_acc = ctx.enter_context(tc.tile_pool(name="accsb", bufs=1))

        acc_sb = pool_acc.tile([P, nacc], F32, tag="acc")
        dma_engines = [nc.sync, nc.scalar] if dma_alt else [nc.sync]
        dma_i = [0]

        def dma(out_ap, in_ap):
            eng = dma_engines[dma_i[0] % len(dma_engines)]
            dma_i[0] += 1
            eng.dma_start(out_ap, in_ap)

        def workload():
            col = 0
            for i in range(IMGS_PER_CORE):
                for h in range(nchunk):
                    sl = bass.ts(h, chunk)
                    if fused_dma:
                        cc = pool_in.tile([P, C, chunk], F32, tag="cc")
                        src = img[i].rearrange("c p w -> p c w")[:, :, sl]
                        dma(cc[:], src)
                        c0, c1, c2 = cc[:, 0, :], cc[:, 1, :], cc[:, 2, :]
                    elif dma_accum:
                        # t = c0 + c1 computed by the SDMA CCE unit:
                        # HWDGE copy of c0, then SWDGE accumulate-add of c1.
                        c0 = pool_in.tile([P, chunk], F32, tag="c0")
                        nc.sync.dma_start(c0[:], img[i, 0, :, sl])
                        nc.gpsimd.dma_start(
                            c0[:], img[i, 1, :, sl], accum_op=add
                        )
                        c2 = pool_in.tile([P, chunk], F32, tag="c2")
                        nc.sync.dma_start(c2[:], img[i, 2, :, sl])
                        c1 = None
                    else:
                        c0 = pool_in.tile([P, chunk], F32, tag="c0")
                        dma(c0[:], img[i, 0, :, sl])
                        c1 = pool_in.tile([P, chunk], F32, tag="c1")
                        dma(c1[:], img[i, 1, :, sl])
                        c2 = pool_in.tile([P, chunk], F32, tag="c2")
                        dma(c2[:], img[i, 2, :, sl])

                    if dma_accum:
                        t = c0
                    else:
                        t = pool_tmp.tile([P, chunk], F32, tag="t")
                        nc.vector.tensor_tensor(t[:], c0[:], c1[:], add)
                    # s = (t + 0.0) + c2, fused row-sum into acc column
                    s = pool_tmp.tile([P, chunk], F32, tag="s")
                    nc.vector.scalar_tensor_tensor(
                        s[:], t[:], 0.0, c2[:], add, add,
                        accum_out=acc_sb[:, col : col + 1],
                    )
                    # bright bits + count; dark-complement bits + count
                    if emit_counts:
                        b1 = pool_bits.tile([P, chunk], F32, tag="bits")
                        nc.vector.tensor_scalar(
                            b1[:], s[:], 2.25, None, is_ge, add,
                            accum_out=acc_sb[:, col + 1 : col + 2],
                        )
                        b2 = pool_bits.tile([P, chunk], F32, tag="bits")
                        nc.vector.tensor_scalar(
                            b2[:], s[:], 0.75, None, is_ge, add,
                            accum_out=acc_sb[:, col + 2 : col + 3],
                        )
                    col += 3

        if reps == 1:
            for _ in range(body_copies):
                workload()
        else:
            with tc.For_i(0, reps, 1):
                for _ in range(body_copies):
                    workload()

        nc.sync.dma_start(acc_out[:, :], acc_sb[:])

    nc.compile()
    return nc, nacc


def _get_nc():
    global _compiled_nc
    if _compiled_nc is None:
        _compiled_nc = _build_bass()[0]
    return _compiled_nc


def kernel(batch_images, base_exposure_1, base_exposure_2):
    global LAST_RESULT
    batch_images = np.ascontiguousarray(np.asarray(batch_images, dtype=np.float32))
    be1 = np.asarray(base_exposure_1, dtype=np.float32)
    be2 = np.asarray(base_exposure_2, dtype=np.float32)
    assert batch_images.shape == (B, C, H, W)

    nc = _get_nc()
    shards = batch_images.reshape(N_CORES, IMGS_PER_CORE, C, P, COLS)
    in_maps = [{"img": shards[c]} for c in range(N_CORES)]
    res = run_bass_kernel_spmd(nc, in_maps, list(range(N_CORES)), trace=TRACE)
    LAST_RESULT = res

    # ---- gather/unshard: fold per-partition partials to per-image stats ----
    sum_s = np.empty(B, dtype=np.float64)
    cnt_bright = np.empty(B, dtype=np.float64)
    cnt_ge_quarter = np.empty(B, dtype=np.float64)
    for c in range(N_CORES):
        acc = np.asarray(res.results[c]["acc"], dtype=np.float64)  # [128, nacc]
        nchunk = acc.shape[1] // (IMGS_PER_CORE * NQ)
        for i in range(IMGS_PER_CORE):
            base = i * nchunk * NQ
            cols = [base + k * NQ for k in range(nchunk)]
            b = c * IMGS_PER_CORE + i
            sum_s[b] = sum(acc[:, j].sum() for j in [cc + 0 for cc in cols])
            cnt_bright[b] = sum(acc[:, j].sum() for j in [cc + 1 for cc in cols])
            cnt_ge_quarter[b] = sum(acc[:, j].sum() for j in [cc + 2 for cc in cols])

    # ---- epilogue: replicate reference numerics in fp32 ----
    f32 = np.float32
    bright = cnt_bright.astype(np.float32)                     # exact counts
    dark = (np.float64(PLANE) - cnt_ge_quarter).astype(np.float32)
    dr = bright / (dark + f32(1e-5))
    bright_avg = (sum_s / 3.0 / PLANE).astype(np.float32)

    g = f32(0.5)
    conds = [
        (dr > f32(1.0)) & (bright_avg > f32(0.4)) & (bright_avg < f32(0.6)),
        bright_avg <= f32(0.3),
        bright_avg >= f32(0.7),
        (dr <= f32(1.0)) & (bright_avg > f32(0.3)) & (bright_avg < f32(0.7)),
    ]
    vals = [g * f32(2.0), g * f32(0.5), g * f32(0.5), g * f32(0.75)]
    gaps = np.select(conds, vals, f32(0.0)).astype(np.float32)

    bl = bright_avg[-1]
    gl = gaps[-1]
    s_ = f32(1.7)
    e1 = np.where(
        bl <= f32(0.25), be1 + f32(0.5) * gl * s_,
        np.where(bl >= f32(0.75), be1 - f32(0.5) * gl * s_, be1 - f32(0.3) * gl),
    ).astype(np.float32)
    e2 = np.where(
        bl <= f32(0.25), be2 + f32(0.5) * gl * s_,
        np.where(bl >= f32(0.75), be2 - f32(0.5) * gl * s_, be2 + f32(0.7) * gl),
    ).astype(np.float32)

    return np.stack([dr, bright_avg, gaps, e1, e2]).astype(np.float32)


# revision 19
# speedup vs baseline: 1.0524x; 1.0524x over previous
"""Trainium2 Bass kernel for nn_CombineModel_wo_net (histogram_binning).

Full inputs in, full output out. Internally: data-parallel across 8
NeuronCores, 2 images per core. Each core streams its 2x3x544x960 fp32
slice from HBM and reduces it to per-partition partials:
  - sum of s = c0+c1+c2 per pixel  (for avg brightness)
  - count(s >= 2.25)               (bright pixels, g >= 0.75)
  - count(s >= 0.75)               (complement of dark: g >= 0.25)
The tiny [5,16] epilogue (dynamic-range ratio, gap select, exposure
where-chains) is replicated exactly in float32 numpy on the host from
the gathered partials.

Threshold equivalence note: comparing s = c0+c1+c2 against 3*T is exact
w.r.t. the reference's g = mean(c) >= T because fp32 spacing at s~3T is
wider than the rounding interval of s/3 (or s*(1/3)) around T for
T in {0.25, 0.75}; no representable s straddles the thresholds.
"""

import sys

for _p in ("/opt/trn_rl_repo",):
    if _p not in sys.path:
        sys.path.insert(0, _p)

from contextlib import ExitStack

import numpy as np

import concourse.bass as bass
import concourse.bacc as bacc
import concourse.mybir as mybir
import concourse.tile as tile
from concourse.bass_utils import run_bass_kernel_spmd

# Problem geometry (hardcoded per contract).
B, C, H, W = 16, 3, 544, 960
N_CORES = 8
IMGS_PER_CORE = B // N_CORES          # 2
PLANE = H * W                          # 522240 = 128 * 4080
P = 128
COLS = PLANE // P                      # 4080
CHUNK = 2040                           # half-plane chunks for DMA/compute overlap
NQ = 3                                 # sum_s, cnt_ge_2.25, cnt_ge_0.75
# Per-image column splits. The last image tapers so that almost no DVE
# work remains after the final DMA byte arrives (tail = ~1.3us instead
# of ~4.4us of STT+2xTS on a full 2040 chunk).
PLAN = [[2040, 2040], [2040, 1020, 612, 408]]
NACC = sum(len(p) for p in PLAN) * NQ  # 18 accumulator columns

F32 = mybir.dt.float32

# Module-level knobs (test.py pokes these; grading path uses defaults).
TRACE = False
LAST_RESULT = None  # BassKernelResults of most recent run (for profiling)

_compiled_nc = None


def _build_bass(reps=1, body_copies=1, chunk=CHUNK, in_bufs=4, tmp_bufs=3,
                bits_bufs=3, emit_counts=True, dma_alt=False, fused_dma=False,
                dma_accum=False, plan=None):
    """Emit the per-core Tile program (same SPMD program on all 8 cores).

    reps > 1 wraps the workload in a hardware For_i loop so one NEFF
    execution runs it `reps * body_copies` times; the bench harness uses
    marginal time per iteration as the HW exec time. The grading path
    uses reps=1, body_copies=1 (no loop).
    """
    if plan is None:
        plan = [[chunk] * (COLS // chunk) for _ in range(IMGS_PER_CORE)]
    nacc = sum(len(p) for p in plan) * NQ
    max_chunk = max(max(p) for p in plan)
    nc = bacc.Bacc(
        "TRN2", target_bir_lowering=False, debug=False, num_devices=N_CORES
    )
    img = nc.dram_tensor(
        "img", [IMGS_PER_CORE, C, P, COLS], F32, kind="ExternalInput"
    ).ap()
    acc_out = nc.dram_tensor("acc", [P, nacc], F32, kind="ExternalOutput").ap()

    add = mybir.AluOpType.add
    is_ge = mybir.AluOpType.is_ge

    with ExitStack() as ctx:
        tc = ctx.enter_context(tile.TileContext(nc))
        pool_in = ctx.enter_context(tc.tile_pool(name="inp", bufs=in_bufs))
        pool_tmp = ctx.enter_context(tc.tile_pool(name="tmp", bufs=tmp_bufs))
        pool_bits = ctx.enter_context(
            tc.tile_pool(name="bitsp", bufs=bits_bufs or tmp_bufs)
        )
        pool_acc = ctx.enter_context(tc.tile_pool(name="accsb", bufs=1))

        acc_sb = pool_acc.tile([P, nacc], F32, tag="acc")

        def workload():
            col = 0
            for i in range(IMGS_PER_CORE):
                start = 0
                for size in plan[i]:
                    sl = slice(start, start + size)
                    start += size
                    c0 = pool_in.tile([P, size], F32, tag="c0")
                    nc.sync.dma_start(c0[:], img[i, 0, :, sl])
                    c1 = pool_in.tile([P, size], F32, tag="c1")
                    nc.sync.dma_start(c1[:], img[i, 1, :, sl])
                    c2 = pool_in.tile([P, size], F32, tag="c2")
                    nc.sync.dma_start(c2[:], img[i, 2, :, sl])

                    t = pool_tmp.tile([P, size], F32, tag="t")
                    nc.vector.tensor_tensor(t[:], c0[:], c1[:], add)
                    # s = (t + 0.0) + c2, fused row-sum into acc column
                    s = pool_tmp.tile([P, size], F32, tag="s")
                    nc.vector.scalar_tensor_tensor(
                        s[:], t[:], 0.0, c2[:], add, add,
                        accum_out=acc_sb[:, col : col + 1],
                    )
                    # bright bits + count; dark-complement bits + count
                    if emit_counts:
                        b1 = pool_bits.tile([P, size], F32, tag="bits")
                        nc.vector.tensor_scalar(
                            b1[:], s[:], 2.25, None, is_ge, add,
                            accum_out=acc_sb[:, col + 1 : col + 2],
                        )
                        b2 = pool_bits.tile([P, size], F32, tag="bits")
                        nc.vector.tensor_scalar(
                            b2[:], s[:], 0.75, None, is_ge, add,
                            accum_out=acc_sb[:, col + 2 : col + 3],
                        )
                    col += 3

        if reps == 1:
            for _ in range(body_copies):
                workload()
        else:
            with tc.For_i(0, reps, 1):
                for _ in range(body_copies):
                    workload()

        nc.sync.dma_start(acc_out[:, :], acc_sb[:])

    nc.compile()
    return nc, nacc


def _get_nc():
    global _compiled_nc
    if _compiled_nc is None:
        _compiled_nc = _build_bass(plan=PLAN)[0]
    return _compiled_nc


def kernel(batch_images, base_exposure_1, base_exposure_2):
    global LAST_RESULT
    batch_images = np.ascontiguousarray(np.asarray(batch_images, dtype=np.float32))
    be1 = np.asarray(base_exposure_1, dtype=np.float32)
    be2 = np.asarray(base_exposure_2, dtype=np.float32)
    assert batch_images.shape == (B, C, H, W)

    nc = _get_nc()
    shards = batch_images.reshape(N_CORES, IMGS_PER_CORE, C, P, COLS)
    in_maps = [{"img": shards[c]} for c in range(N_CORES)]
    res = run_bass_kernel_spmd(nc, in_maps, list(range(N_CORES)), trace=TRACE)
    LAST_RESULT = res

    # ---- gather/unshard: fold per-partition partials to per-image stats ----
    sum_s = np.empty(B, dtype=np.float64)
    cnt_bright = np.empty(B, dtype=np.float64)
    cnt_ge_quarter = np.empty(B, dtype=np.float64)
    for c in range(N_CORES):
        acc = np.asarray(res.results[c]["acc"], dtype=np.float64)  # [128, NACC]
        col = 0
        for i, sizes in enumerate(PLAN):
            cols = [col + k * NQ for k in range(len(sizes))]
            col += len(sizes) * NQ
            b = c * IMGS_PER_CORE + i
            sum_s[b] = sum(acc[:, j].sum() for j in [cc + 0 for cc in cols])
            cnt_bright[b] = sum(acc[:, j].sum() for j in [cc + 1 for cc in cols])
            cnt_ge_quarter[b] = sum(acc[:, j].sum() for j in [cc + 2 for cc in cols])

    # ---- epilogue: replicate reference numerics in fp32 ----
    f32 = np.float32
    bright = cnt_bright.astype(np.float32)                     # exact counts
    dark = (np.float64(PLANE) - cnt_ge_quarter).astype(np.float32)
    dr = bright / (dark + f32(1e-5))
    bright_avg = (sum_s / 3.0 / PLANE).astype(np.float32)

    g = f32(0.5)
    conds = [
        (dr > f32(1.0)) & (bright_avg > f32(0.4)) & (bright_avg < f32(0.6)),
        bright_avg <= f32(0.3),
        bright_avg >= f32(0.7),
        (dr <= f32(1.0)) & (bright_avg > f32(0.3)) & (bright_avg < f32(0.7)),
    ]
    vals = [g * f32(2.0), g * f32(0.5), g * f32(0.5), g * f32(0.75)]
    gaps = np.select(conds, vals, f32(0.0)).astype(np.float32)

    bl = bright_avg[-1]
    gl = gaps[-1]
    s_ = f32(1.7)
    e1 = np.where(
        bl <= f32(0.25), be1 + f32(0.5) * gl * s_,
        np.where(bl >= f32(0.75), be1 - f32(0.5) * gl * s_, be1 - f32(0.3) * gl),
    ).astype(np.float32)
    e2 = np.where(
        bl <= f32(0.25), be2 + f32(0.5) * gl * s_,
        np.where(bl >= f32(0.75), be2 - f32(0.5) * gl * s_, be2 + f32(0.7) * gl),
    ).astype(np.float32)

    return np.stack([dr, bright_avg, gaps, e1, e2]).astype(np.float32)
